# revision 24
# baseline (speedup 1.0000x reference)
"""Trainium2 Bass kernel for nn_BlockAttnResRouter (moe_routing).

Reference computation (n=8, b=4, t=2048, h=2048, position=n):
    keys   = rms_norm(values) + key_pos_bias[:n]            # per (n,b,t) row over h
    score  = (q . keys) / sqrt(h)                            # q = w_query[position]
    alpha  = softmax(score, axis=n)
    routed = sum_n alpha[n] * values[n]                      # [b,t,h]
    returns (routed, alpha transposed to [b,t,n])

Key algebraic simplification used on-device:
    score[n,tok] = (inv[n,tok] * (q.v[n,tok]) + c_n) / sqrt(h)
    inv          = rsqrt(mean(v^2) + eps)  computed as exp(-0.5*ln(mean+eps))
    c_n          = q . key_pos_bias[n]     (precomputed host-side, tiny)

Sharding: split along t across the 8 cores (each core: [8, 1024, 2048] slab,
tokens = flattened (b, t_local)).  All reductions are over n and h -> local.

Per-core layout: v tiles [128, 2048] with partition p = (tok%16)*8 + n, so each
DMA moves 128 contiguous 8 KiB rows.  Engine assignment:
  - ScalarE: Square+accum (sumsq), Ln/Exp (rsqrt + softmax exp), PSUM->SBUF copy
  - VectorE: tensor_tensor_reduce (q.v), tiny softmax ops, masked-weight builds
  - TensorE: 8-way softmax sums/broadcast via 0/1 selector matmuls; the
    weighted sum  routed = sum_n alpha*v  as block-diagonal masked matmuls
    accumulating a [128, 2048] PSUM tile (contraction over partitions).
"""

import sys

if "/opt/trn_rl_repo" not in sys.path:
    sys.path.insert(0, "/opt/trn_rl_repo")

from contextlib import ExitStack

import numpy as np

import concourse.bass as bass
import concourse.tile as tile
from concourse import bacc, mybir
from concourse.bass import ts

# ---- problem constants (hardcoded per harness contract) ----
N = 8           # number of sources (= position)
B = 4
T_FULL = 2048
H = 2048
N_CORES = 8
T_CORE = B * T_FULL // N_CORES      # 1024 tokens per core
TPT = 16                            # tokens per v-tile (x8 n = 128 partitions)
N_TILES = T_CORE // TPT             # 64 v-tiles per core
TILES_PER_GROUP = 8                 # one softmax batch / routed group = 128 tokens
N_GROUPS = N_TILES // TILES_PER_GROUP   # 8
EPS = 1e-6
SCALE = float(np.sqrt(H))           # * TEMPERATURE (=1.0)

F32 = mybir.dt.float32
F32R = mybir.dt.float32r   # fp32 bytes, TF32-class single-pass PE matmul
BF16 = mybir.dt.bfloat16
AF = mybir.ActivationFunctionType
OP = mybir.AluOpType

PACK = 4                   # v-tiles per input DMA (4 MiB transfers)

_PROGRAM = None


def _patch_act_tables(nc):
    """Make natural_log_exp_and_others the only set offering Square/Ln/Exp so
    the table-load inserter emits ONE load instead of ping-ponging between the
    natural_log and exp_and_others sets every group (17 loads, ~2.7us each).
    Keys/order are preserved, so act_func_set_id stays valid for the runtime."""
    import concourse.bacc as bacc_mod
    from concourse.hw_specs import get_activation_tables as orig_get

    want = {AF.Square, AF.Ln, AF.Exp}
    keep = "natural_log_exp_and_others"

    def patched(arch):
        orig = orig_get(arch)
        return {
            name: (set(fns) if name == keep else set(fns) - want)
            for name, fns in orig.items()
        }

    bacc_mod.get_activation_tables = patched


def build_program(repeats=1):
    """repeats>1 duplicates the whole body inside one NEFF (idempotent —
    same inputs/outputs each pass); used only for wall-clock calibration."""
    nc = bacc.Bacc(None)
    _patch_act_tables(nc)

    # vals pre-interleaved host-side: vals[k, t*8+n, :] = v[n, 16k+t, :]
    # float32r tag = same fp32 bytes, single-pass (TF32-ish) PE matmul
    vals = nc.declare_dram_parameter("vals", [N_TILES // PACK, 128, PACK, H],
                                     F32R, isOutput=False)
    qbc_d = nc.declare_dram_parameter("qbc", [128, H], F32, isOutput=False)
    cb_d = nc.declare_dram_parameter("cb", [128, 1], F32, isOutput=False)
    masks_d = nc.declare_dram_parameter("masks", [128, TILES_PER_GROUP, 128], F32, isOutput=False)
    s1_d = nc.declare_dram_parameter("s1", [128, TPT], F32, isOutput=False)
    s2_d = nc.declare_dram_parameter("s2", [TPT, 128], F32, isOutput=False)
    id_d = nc.declare_dram_parameter("ident", [128, 128], F32, isOutput=False)
    routed_d = nc.declare_dram_parameter("routed", [T_CORE, H], F32, isOutput=True)
    # alpha as [64, 128]: row k holds tokens [16k,16k+16) x n in (t,n) order,
    # which is exactly contiguous [T_CORE, N] row-major.
    alpha_d = nc.declare_dram_parameter("alpha", [N_TILES, 128], F32, isOutput=True)

    with tile.TileContext(nc) as tc, ExitStack() as ctx:
        singles = ctx.enter_context(tc.tile_pool(name="singles", bufs=1))
        vpool = ctx.enter_context(tc.tile_pool(name="vpool", bufs=4))
        sqscr_pool = ctx.enter_context(tc.tile_pool(name="sqscr", bufs=2))
        qvscr_pool = ctx.enter_context(tc.tile_pool(name="qvscr", bufs=2))
        rstage_pool = ctx.enter_context(tc.tile_pool(name="rstage", bufs=2))
        stats = ctx.enter_context(tc.tile_pool(name="stats", bufs=3))
        lhs_pool = ctx.enter_context(tc.tile_pool(name="lhs", bufs=10))
        pbig = ctx.enter_context(tc.tile_pool(name="pbig", bufs=2, space="PSUM"))
        psmall = ctx.enter_context(tc.tile_pool(name="psmall", bufs=1, space="PSUM"))

        qbc = singles.tile([128, H], F32)
        nc.sync.dma_start(out=qbc, in_=qbc_d[:])
        cb = singles.tile([128, 1], F32)
        nc.sync.dma_start(out=cb, in_=cb_d[:])
        masks = singles.tile([128, TILES_PER_GROUP, 128], F32)
        nc.sync.dma_start(out=masks, in_=masks_d[:])
        s1 = singles.tile([128, TPT], F32)
        nc.sync.dma_start(out=s1, in_=s1_d[:])
        s2 = singles.tile([TPT, 128], F32)
        nc.sync.dma_start(out=s2, in_=s2_d[:])
        ident = singles.tile([128, 128], F32)
        nc.sync.dma_start(out=ident, in_=id_d[:])
        # alpha staging: [8 partitions = tile-within-group, groups x 128]
        alpha_stage = singles.tile([TILES_PER_GROUP, N_GROUPS, 128], F32)
        eps_t = singles.tile([128, 1], F32)
        nc.vector.memset(eps_t, EPS)
        zero_t = singles.tile([128, 1], F32)
        nc.vector.memset(zero_t, 0.0)

        for g_rep in range(N_GROUPS * repeats):
            g = g_rep % N_GROUPS
            sq_b = stats.tile([128, TILES_PER_GROUP], F32)
            qv_b = stats.tile([128, TILES_PER_GROUP], F32)
            vts = []  # per tile j in group: (pack_tile, slot) -> AP
            for half in range(TILES_PER_GROUP // PACK):
                kp = (TILES_PER_GROUP * g) // PACK + half
                vp = vpool.tile([128, PACK, H], F32R)
                nc.sync.dma_start(out=vp, in_=vals[kp])
                for s in range(PACK):
                    j = half * PACK + s
                    vt = vp[:, s, :]
                    vts.append(vt)
                    sscr = sqscr_pool.tile([128, H], BF16)
                    nc.scalar.activation(
                        out=sscr, in_=vt.bitcast(F32), func=AF.Square,
                        accum_out=sq_b[:, j : j + 1],
                    )
                    qscr = qvscr_pool.tile([128, H], BF16)
                    nc.vector.scalar_tensor_tensor(
                        out=qscr, in0=vt.bitcast(F32), scalar=1.0, in1=qbc,
                        op0=OP.mult, op1=OP.mult,
                        accum_out=qv_b[:, j : j + 1],
                    )

            # softmax over n (8 strided partitions) for 8 tiles at once
            l_b = stats.tile([128, TILES_PER_GROUP], F32)
            nc.scalar.activation(out=l_b, in_=sq_b, func=AF.Ln,
                                 scale=1.0 / H, bias=eps_t[:, 0:1])
            inv_b = stats.tile([128, TILES_PER_GROUP], F32)
            nc.scalar.activation(out=inv_b, in_=l_b, func=AF.Exp, scale=-0.5,
                                 bias=zero_t[:, 0:1])
            t_b = stats.tile([128, TILES_PER_GROUP], F32)
            nc.vector.tensor_mul(t_b, inv_b, qv_b)
            e_b = stats.tile([128, TILES_PER_GROUP], F32)
            nc.scalar.activation(out=e_b, in_=t_b, func=AF.Exp,
                                 scale=1.0 / SCALE, bias=cb[:, 0:1])
            sums_ps = psmall.tile([TPT, TILES_PER_GROUP], F32)
            nc.tensor.matmul(sums_ps, s1, e_b, start=True, stop=True)
            recip_b = stats.tile([TPT, TILES_PER_GROUP], F32)
            nc.vector.reciprocal(out=recip_b, in_=sums_ps)
            bc_ps = psmall.tile([128, TILES_PER_GROUP], F32)
            nc.tensor.matmul(bc_ps, s2, recip_b, start=True, stop=True)
            alpha_b = stats.tile([128, TILES_PER_GROUP], F32)
            nc.vector.tensor_mul(alpha_b, e_b, bc_ps)

            # alpha output: transpose [128, 8] -> [8, 128] and stage
            at_ps = psmall.tile([TILES_PER_GROUP, 128], F32)
            nc.tensor.transpose(at_ps, alpha_b, ident)
            nc.vector.tensor_copy(alpha_stage[:, g, :], at_ps)

            # routed: block-diagonal masked matmuls accumulating PSUM.
            # Two [128, 1024] half-tiles (2 banks each, double-buffered) so
            # ScalarE evacuation overlaps the next half/group's matmuls.
            lts = []
            for j in range(TILES_PER_GROUP):
                lt = lhs_pool.tile([128, 128], F32R)
                nc.vector.tensor_scalar_mul(
                    out=lt, in0=masks[:, j, :], scalar1=alpha_b[:, j : j + 1]
                )
                lts.append(lt)
            for hh in range(2):
                rps = pbig.tile([128, H // 2], F32)
                for j in range(TILES_PER_GROUP):
                    for c in range(2):
                        nc.tensor.matmul(
                            rps[:, ts(c, 512)], lts[j],
                            vts[j][:, ts(2 * hh + c, 512)],
                            start=(j == 0), stop=(j == TILES_PER_GROUP - 1),
                        )
                rst = rstage_pool.tile([128, H // 2], F32)
                nc.scalar.copy(out=rst, in_=rps)
                nc.sync.dma_start(
                    out=routed_d[ts(g, 128), ts(hh, H // 2)], in_=rst
                )

        nc.sync.dma_start(
            out=alpha_d[:].rearrange("(g j) m -> j g m", j=TILES_PER_GROUP),
            in_=alpha_stage,
        )

    nc.compile()
    return nc


def get_program():
    global _PROGRAM
    if _PROGRAM is None:
        _PROGRAM = build_program()
    return _PROGRAM


def make_host_inputs(w_query, key_pos_bias, position):
    """Tiny precomputed constant tensors shared by all cores."""
    pos = int(position)
    q = np.ascontiguousarray(w_query[pos].astype(np.float32))          # [H]
    c = key_pos_bias[:N].astype(np.float32) @ q                        # [N]
    qbc = np.ascontiguousarray(np.broadcast_to(q, (128, H)))
    cb = np.tile(c / SCALE, TPT).astype(np.float32).reshape(128, 1)    # p = t*8+n -> c[p%8]
    p = np.arange(128)
    s1 = (p[:, None] // N == np.arange(TPT)[None, :]).astype(np.float32)
    s2 = np.ascontiguousarray(s1.T)
    masks = np.zeros((TILES_PER_GROUP, 128, 128), dtype=np.float32)
    for kl in range(TILES_PER_GROUP):
        masks[kl, p, kl * TPT + p // N] = 1.0
    masks = np.ascontiguousarray(masks.transpose(1, 0, 2))  # [p, k_local, m]
    ident = np.eye(128, dtype=np.float32)
    return dict(qbc=qbc, cb=cb, masks=masks, s1=s1, s2=s2, ident=ident)


def shard_values(values):
    """[N, B, T_FULL, H] -> 8 slabs [N_TILES//PACK, 128, PACK, H]: t-sharded,
    interleaved so slab[kp, t*8+n, s, :] = v[n, tok=16*(PACK*kp+s)+t, :] with
    tok the core-local flattened (b, t_local) token index."""
    n, b, t, h = values.shape
    tl = t // N_CORES
    v5 = values.reshape(n, b, N_CORES, tl, h)
    out = []
    for i in range(N_CORES):
        s = v5[:, :, i].reshape(n, N_TILES // PACK, PACK, TPT, h)  # n,kp,s,t,h
        out.append(np.ascontiguousarray(
            s.transpose(1, 3, 0, 2, 4)                  # kp, t, n, s, h
             .reshape(N_TILES // PACK, 128, PACK, h)))
    return out


def kernel(values, w_query, key_pos_bias, position):
    values = np.asarray(values)
    n, b, t, h = values.shape
    assert (n, b, t, h) == (N, B, T_FULL, H), f"unexpected shape {values.shape}"

    nc = get_program()
    const = make_host_inputs(np.asarray(w_query), np.asarray(key_pos_bias), position)
    shards = shard_values(values)
    in_maps = [dict(vals=shards[i], **const) for i in range(N_CORES)]

    from concourse.bass_utils import run_bass_kernel_spmd

    res = run_bass_kernel_spmd(nc, in_maps, core_ids=list(range(N_CORES)))

    tl = T_FULL // N_CORES
    routed = np.empty((B, T_FULL, H), dtype=np.float32)
    alpha = np.empty((B, T_FULL, N), dtype=np.float32)
    rv = routed.reshape(B, N_CORES, tl, H)
    av = alpha.reshape(B, N_CORES, tl, N)
    for i in range(N_CORES):
        r = np.asarray(res.results[i]["routed"]).reshape(B, tl, H)
        a = np.asarray(res.results[i]["alpha"]).reshape(B, tl, N)
        rv[:, i] = r
        av[:, i] = a
    return routed, alpha
